# revision 22
# baseline (speedup 1.0000x reference)
"""Trainium2 Bass kernel: single attention head (B=8, S=2048, E=1024, H=64).

Sharding: data-parallel over batch -- each of the 8 NeuronCores computes one
batch element's full attention. No collectives needed; every HBM byte is read
exactly once chip-wide.

Per-core pipeline (one batch element):
  - Inputs are staged host-side as X^T ([E, S], contiguous) so the contraction
    dim lands on SBUF partitions with perfectly contiguous DMA.
  - fp16 compute: X chunks are cast f32->f16 during the SWDGE DMA.
  - Projections q^T/k^T/v^T = W^T @ X^T on TensorE, PSUM-accumulated over 8
    K-chunks of 128. bq folds into q^T during PSUM evacuation; bk cancels in
    softmax (adds a per-query constant to every score); bv folds into v.
  - Scores are computed TRANSPOSED: S^T[sk, sq] = k^T.T @ q^T, so softmax's
    sum runs over the partition axis, which we get for free by augmenting v
    with a ones column: [v | 1].T @ exp(S^T) yields [out^T ; rowsums].
  - exp on ScalarE (fp32 PSUM in -> fp16 SBUF out), scale=1/8 fused.
  - v-projection matmuls are interleaved into the scores stream so the PE
    FIFO never blocks the exp chain on late V DMA chunks.
  - AV accumulates over all 16 key tiles into one PSUM tile [65, 2048] that
    reuses the v-projection PSUM banks.
  - Finalize: transpose 128-column chunks via TensorE, divide by the rowsum
    column with VectorE reciprocal + tensor_scalar, batched fp32 DMAs out.
"""

import numpy as np

import concourse.bass as bass  # noqa: F401  (engine namespaces live on nc)
import concourse.mybir as mybir
import concourse.tile as tile
from concourse import bacc
from concourse.bass_utils import run_bass_kernel_spmd
from concourse.masks import make_identity

B, S, E, H = 8, 2048, 1024, 64
EC = E // 128   # contraction chunks per projection
NT = S // 128   # key tiles
F16 = mybir.dt.float16
F32 = mybir.dt.float32

_CACHE = {}


def _build_nc():
    nc = bacc.Bacc(None)
    xq = nc.declare_dram_parameter("xqt", [E, S], F32, isOutput=False)
    xk = nc.declare_dram_parameter("xkt", [E, S], F32, isOutput=False)
    xv = nc.declare_dram_parameter("xvt", [E, S], F32, isOutput=False)
    wq = nc.declare_dram_parameter("wq", [E, H], F32, isOutput=False)
    wk = nc.declare_dram_parameter("wk", [E, H], F32, isOutput=False)
    wv = nc.declare_dram_parameter("wv", [E, H], F32, isOutput=False)
    bq = nc.declare_dram_parameter("bq", [H, 1], F32, isOutput=False)
    bv = nc.declare_dram_parameter("bv", [H, 1], F32, isOutput=False)
    out = nc.declare_dram_parameter("out", [S, H], F32, isOutput=True)

    Exp = mybir.ActivationFunctionType.Exp

    with tile.TileContext(nc) as tc:
        with tc.tile_pool(name="const", bufs=1) as const, \
             tc.tile_pool(name="xio", bufs=10) as xio, \
             tc.tile_pool(name="ptp", bufs=NT) as ptp, \
             tc.tile_pool(name="p5sb", bufs=2) as p5sb:

            # weights: one casting SWDGE DMA each, at the head of the queue
            wts = {}
            for nm, dram in (("q", wq), ("k", wk), ("v", wv)):
                wt = const.tile([128, EC, H], F16, name=f"w{nm}")
                nc.gpsimd.dma_start(
                    out=wt[:], in_=dram[:].rearrange("(c p) h -> p c h", p=128))
                wts[nm] = wt
            bq_t = const.tile([H, 1], F32, name="bq_t")
            nc.sync.dma_start(out=bq_t[:], in_=bq[:])
            bv_t = const.tile([H, 1], F32, name="bv_t")
            nc.sync.dma_start(out=bv_t[:], in_=bv[:])

            qt = const.tile([64, S], F16, name="qt")
            kt = const.tile([64, S], F16, name="kt")
            vt = const.tile([64, S], F16, name="vt")
            vaug = const.tile([128, NT, 80], F16, name="vaug")
            oasb = const.tile([65, S], F16, name="oasb")
            ident = const.tile([128, 128], F16, name="ident")
            osb_all = const.tile([128, NT, H], F32, name="osb_all")

            def proj(nm, xdram, ps):
                for c in range(EC):
                    xt_ = xio.tile([128, S], F16, tag="xt", name=f"x{nm}{c}")
                    nc.gpsimd.dma_start(out=xt_[:], in_=xdram[c * 128:(c + 1) * 128, :])
                    for n in range(S // 512):
                        nc.tensor.matmul(
                            ps[:, n * 512:(n + 1) * 512],
                            wts[nm][:, c, :], xt_[:, n * 512:(n + 1) * 512],
                            start=(c == 0), stop=(c == EC - 1))

            # q/k projections (PSUM: 2 x 4 banks)
            with tc.tile_pool(name="ppsum", bufs=2, space="PSUM") as pp:
                psq = pp.tile([64, S], F32, tag="proj", name="psq")
                proj("q", xq, psq[:])
                nc.vector.tensor_scalar_add(qt[:], psq[:], bq_t[:])
                psk = pp.tile([64, S], F32, tag="proj", name="psk")
                proj("k", xk, psk[:])
                nc.vector.tensor_copy(kt[:], psk[:])

            # attention phase (PSUM: aux 4 banks + scores 2x2 banks)
            with tc.tile_pool(name="aux", bufs=1, space="PSUM") as auxp, \
                 tc.tile_pool(name="spsum", bufs=2, space="PSUM") as sps:
                vps = auxp.tile([65, S], F32, tag="big", name="vps")
                pts = [None] * NT

                def v_chunk(c):
                    xt_ = xio.tile([128, S], F16, tag="xt", name=f"xv{c}")
                    nc.gpsimd.dma_start(out=xt_[:], in_=xv[c * 128:(c + 1) * 128, :])
                    for n in range(S // 512):
                        nc.tensor.matmul(
                            vps[0:64, n * 512:(n + 1) * 512],
                            wts["v"][:, c, :], xt_[:, n * 512:(n + 1) * 512],
                            start=(c == 0), stop=(c == EC - 1))

                def s_tile(t):
                    pt_ = ptp.tile([128, S], F16, tag="pt", name=f"pt{t}")
                    pts[t] = pt_
                    for h2 in range(2):
                        st = sps.tile([128, 1024], F32, tag="st", name=f"st{t}_{h2}")
                        for n in range(2):
                            nc.tensor.matmul(
                                st[:, n * 512:(n + 1) * 512],
                                kt[:, t * 128:(t + 1) * 128],
                                qt[:, h2 * 1024 + n * 512: h2 * 1024 + (n + 1) * 512],
                                start=True, stop=True)
                        nc.scalar.activation(
                            pt_[:, h2 * 1024:(h2 + 1) * 1024], st[:], Exp, scale=0.125)

                def av_one(t, cq):
                    nc.tensor.matmul(
                        oa[:, cq * 512:(cq + 1) * 512],
                        vaug[:, t, 0:65], pts[t][:, cq * 512:(cq + 1) * 512],
                        start=(t == 0), stop=(t == NT - 1),
                        skip_group_check=True)

                def av_tile(t):
                    for cq in range(4):
                        av_one(t, cq)

                # Interleave v-projection chunks into the scores stream at
                # positions where their DMA has already landed, so they never
                # stall the PE FIFO ahead of score matmuls feeding ScalarE.
                s_tile(0)
                s_tile(1)
                s_tile(2)
                for c in range(7):
                    v_chunk(c)
                    s_tile(3 + c)
                v_chunk(7)
                nc.vector.tensor_scalar_add(vt[:], vps[0:64, :], bv_t[:])
                nc.vector.memset(vaug[:, :, 64], 1.0)
                nc.sync.dma_start_transpose(vaug[:, :, 0:64], vt[:])
                make_identity(nc, ident[:])  # late: keeps GpSimd queue clear

                s_tile(10)
                s_tile(11)
                oa = auxp.tile([65, S], F32, tag="big", name="oa")
                for t in range(12, NT):
                    s_tile(t)
                    av_tile(2 * (t - 12))
                    av_tile(2 * (t - 12) + 1)
                for t in range(8, NT):
                    av_tile(t)
                for cq in range(4):
                    nc.vector.tensor_copy(
                        oasb[:, cq * 512:(cq + 1) * 512],
                        oa[:, cq * 512:(cq + 1) * 512])

            # finalize: transpose, normalize, store (PSUM: 2 x 1 bank)
            out_r = out[:].rearrange("(t p) h -> p t h", p=128)
            with tc.tile_pool(name="p5ps", bufs=2, space="PSUM") as p5ps:
                for cq in range(4):
                    for jj in range(4):
                        j = cq * 4 + jj
                        tr = p5ps.tile([128, 65], F16, tag="tr", name=f"tr{j}")
                        nc.tensor.transpose(
                            tr[:], oasb[:, j * 128:(j + 1) * 128], ident[0:65, 0:65])
                        rc = p5sb.tile([128, 1], F32, tag="rc", name=f"rc{j}")
                        nc.vector.reciprocal(rc[:], tr[:, 64:65])
                        nc.vector.tensor_scalar(
                            osb_all[:, j, :], tr[:, 0:64], rc[:], None,
                            op0=mybir.AluOpType.mult)
                    nc.sync.dma_start(
                        out=out_r[:, cq * 4:(cq + 1) * 4, :],
                        in_=osb_all[:, cq * 4:(cq + 1) * 4, :])

    nc.finalize()
    return nc


def get_nc():
    if "nc" not in _CACHE:
        _CACHE["nc"] = _build_nc()
    return _CACHE["nc"]


def make_in_maps(inputs):
    q = np.asarray(inputs["query"], np.float32)
    k = np.asarray(inputs["key_"], np.float32)
    v = np.asarray(inputs["value"], np.float32)
    wq = np.ascontiguousarray(np.asarray(inputs["Wq"], np.float32))
    wk = np.ascontiguousarray(np.asarray(inputs["Wk"], np.float32))
    wv = np.ascontiguousarray(np.asarray(inputs["Wv"], np.float32))
    bq = np.ascontiguousarray(np.asarray(inputs["bq"], np.float32).reshape(H, 1))
    bv = np.ascontiguousarray(np.asarray(inputs["bv"], np.float32).reshape(H, 1))
    in_maps = []
    for b in range(B):
        in_maps.append({
            "xqt": np.ascontiguousarray(q[b].T),
            "xkt": np.ascontiguousarray(k[b].T),
            "xvt": np.ascontiguousarray(v[b].T),
            "wq": wq, "wk": wk, "wv": wv,
            "bq": bq, "bv": bv,
        })
    return in_maps


def kernel(**inputs):
    nc = get_nc()
    in_maps = make_in_maps(inputs)
    res = run_bass_kernel_spmd(nc, in_maps, list(range(B)))
    return np.stack([res.results[b]["out"] for b in range(B)], axis=0)


# revision 23
# speedup vs baseline: 1.0143x; 1.0143x over previous
"""Trainium2 Bass kernel: single attention head (B=8, S=2048, E=1024, H=64).

Sharding: data-parallel over batch -- each of the 8 NeuronCores computes one
batch element's full attention. No collectives needed; every HBM byte is read
exactly once chip-wide.

Per-core pipeline (one batch element):
  - Inputs are staged host-side as X^T ([E, S], contiguous) so the contraction
    dim lands on SBUF partitions with perfectly contiguous DMA.
  - fp16 compute: X chunks are cast f32->f16 during the SWDGE DMA.
  - Projections q^T/k^T/v^T = W^T @ X^T on TensorE, PSUM-accumulated over 8
    K-chunks of 128. bq folds into q^T during PSUM evacuation; bk cancels in
    softmax (adds a per-query constant to every score); bv folds into v.
  - Scores are computed TRANSPOSED: S^T[sk, sq] = k^T.T @ q^T, so softmax's
    sum runs over the partition axis, which we get for free by augmenting v
    with a ones column: [v | 1].T @ exp(S^T) yields [out^T ; rowsums].
  - exp on ScalarE (fp32 PSUM in -> fp16 SBUF out), scale=1/8 fused.
  - v-projection matmuls are interleaved into the scores stream so the PE
    FIFO never blocks the exp chain on late V DMA chunks.
  - AV accumulates over all 16 key tiles into one PSUM tile [65, 2048] that
    reuses the v-projection PSUM banks.
  - Finalize: transpose 128-column chunks via TensorE, divide by the rowsum
    column with VectorE reciprocal + tensor_scalar, batched fp32 DMAs out.
"""

import numpy as np

import concourse.bass as bass  # noqa: F401  (engine namespaces live on nc)
import concourse.mybir as mybir
import concourse.tile as tile
from concourse import bacc
from concourse.bass_utils import run_bass_kernel_spmd
from concourse.masks import make_identity

B, S, E, H = 8, 2048, 1024, 64
EC = E // 128   # contraction chunks per projection
NT = S // 128   # key tiles
F16 = mybir.dt.float16
F32 = mybir.dt.float32

_CACHE = {}


def _build_nc():
    nc = bacc.Bacc(None)
    xq = nc.declare_dram_parameter("xqt", [E, S], F32, isOutput=False)
    xk = nc.declare_dram_parameter("xkt", [E, S], F32, isOutput=False)
    xv = nc.declare_dram_parameter("xvt", [E, S], F32, isOutput=False)
    wq = nc.declare_dram_parameter("wq", [E, H], F32, isOutput=False)
    wk = nc.declare_dram_parameter("wk", [E, H], F32, isOutput=False)
    wv = nc.declare_dram_parameter("wv", [E, H], F32, isOutput=False)
    bq = nc.declare_dram_parameter("bq", [H, 1], F32, isOutput=False)
    bv = nc.declare_dram_parameter("bv", [H, 1], F32, isOutput=False)
    out = nc.declare_dram_parameter("out", [S, H], F32, isOutput=True)

    Exp = mybir.ActivationFunctionType.Exp

    with tile.TileContext(nc) as tc:
        with tc.tile_pool(name="const", bufs=1) as const, \
             tc.tile_pool(name="xio", bufs=10) as xio, \
             tc.tile_pool(name="ptp", bufs=NT) as ptp, \
             tc.tile_pool(name="p5sb", bufs=2) as p5sb:

            # weights: one casting SWDGE DMA each, at the head of the queue
            wts = {}
            for nm, dram in (("q", wq), ("k", wk), ("v", wv)):
                wt = const.tile([128, EC, H], F16, name=f"w{nm}")
                nc.gpsimd.dma_start(
                    out=wt[:], in_=dram[:].rearrange("(c p) h -> p c h", p=128))
                wts[nm] = wt
            bq_t = const.tile([H, 1], F32, name="bq_t")
            nc.sync.dma_start(out=bq_t[:], in_=bq[:])
            bv_t = const.tile([H, 1], F32, name="bv_t")
            nc.sync.dma_start(out=bv_t[:], in_=bv[:])
            # dummy exp: forces ScalarE's ~2.7us exp table load to happen here,
            # during the DMA phase, instead of right before the first real exp
            dume = const.tile([H, 1], F16, name="dume")
            nc.scalar.activation(dume[:], bq_t[:], Exp)

            qt = const.tile([64, S], F16, name="qt")
            kt = const.tile([64, S], F16, name="kt")
            vt = const.tile([64, S], F16, name="vt")
            vaug = const.tile([128, NT, 80], F16, name="vaug")
            oasb = const.tile([65, S], F16, name="oasb")
            ident = const.tile([128, 128], F16, name="ident")
            osb_all = const.tile([128, NT, H], F32, name="osb_all")

            def proj(nm, xdram, ps):
                for c in range(EC):
                    xt_ = xio.tile([128, S], F16, tag="xt", name=f"x{nm}{c}")
                    nc.gpsimd.dma_start(out=xt_[:], in_=xdram[c * 128:(c + 1) * 128, :])
                    for n in range(S // 512):
                        nc.tensor.matmul(
                            ps[:, n * 512:(n + 1) * 512],
                            wts[nm][:, c, :], xt_[:, n * 512:(n + 1) * 512],
                            start=(c == 0), stop=(c == EC - 1))

            # q/k projections (PSUM: 2 x 4 banks)
            with tc.tile_pool(name="ppsum", bufs=2, space="PSUM") as pp:
                psq = pp.tile([64, S], F32, tag="proj", name="psq")
                proj("q", xq, psq[:])
                nc.vector.tensor_scalar_add(qt[:], psq[:], bq_t[:])
                psk = pp.tile([64, S], F32, tag="proj", name="psk")
                proj("k", xk, psk[:])
                nc.vector.tensor_copy(kt[:], psk[:])

            # attention phase (PSUM: aux 4 banks + scores 2x2 banks)
            with tc.tile_pool(name="aux", bufs=1, space="PSUM") as auxp, \
                 tc.tile_pool(name="spsum", bufs=2, space="PSUM") as sps:
                vps = auxp.tile([65, S], F32, tag="big", name="vps")
                pts = [None] * NT

                def v_chunk(c):
                    xt_ = xio.tile([128, S], F16, tag="xt", name=f"xv{c}")
                    nc.gpsimd.dma_start(out=xt_[:], in_=xv[c * 128:(c + 1) * 128, :])
                    for n in range(S // 512):
                        nc.tensor.matmul(
                            vps[0:64, n * 512:(n + 1) * 512],
                            wts["v"][:, c, :], xt_[:, n * 512:(n + 1) * 512],
                            start=(c == 0), stop=(c == EC - 1))

                def s_tile(t):
                    pt_ = ptp.tile([128, S], F16, tag="pt", name=f"pt{t}")
                    pts[t] = pt_
                    for h2 in range(2):
                        st = sps.tile([128, 1024], F32, tag="st", name=f"st{t}_{h2}")
                        for n in range(2):
                            nc.tensor.matmul(
                                st[:, n * 512:(n + 1) * 512],
                                kt[:, t * 128:(t + 1) * 128],
                                qt[:, h2 * 1024 + n * 512: h2 * 1024 + (n + 1) * 512],
                                start=True, stop=True)
                        nc.scalar.activation(
                            pt_[:, h2 * 1024:(h2 + 1) * 1024], st[:], Exp, scale=0.125)

                def av_one(t, cq):
                    nc.tensor.matmul(
                        oa[:, cq * 512:(cq + 1) * 512],
                        vaug[:, t, 0:65], pts[t][:, cq * 512:(cq + 1) * 512],
                        start=(t == 0), stop=(t == NT - 1),
                        skip_group_check=True)

                def av_tile(t):
                    for cq in range(4):
                        av_one(t, cq)

                # Interleave v-projection chunks into the scores stream at
                # positions where their DMA has already landed, so they never
                # stall the PE FIFO ahead of score matmuls feeding ScalarE.
                s_tile(0)
                s_tile(1)
                s_tile(2)
                for c in range(7):
                    v_chunk(c)
                    s_tile(3 + c)
                v_chunk(7)
                nc.vector.tensor_scalar_add(vt[:], vps[0:64, :], bv_t[:])
                nc.vector.memset(vaug[:, :, 64], 1.0)
                nc.sync.dma_start_transpose(vaug[:, :, 0:64], vt[:])
                make_identity(nc, ident[:])  # late: keeps GpSimd queue clear

                s_tile(10)
                s_tile(11)
                oa = auxp.tile([65, S], F32, tag="big", name="oa")
                for t in range(12, NT):
                    s_tile(t)
                    av_tile(2 * (t - 12))
                    av_tile(2 * (t - 12) + 1)
                for t in range(8, NT):
                    av_tile(t)
                for cq in range(4):
                    nc.vector.tensor_copy(
                        oasb[:, cq * 512:(cq + 1) * 512],
                        oa[:, cq * 512:(cq + 1) * 512])

            # finalize: transpose, normalize, store (PSUM: 2 x 1 bank)
            out_r = out[:].rearrange("(t p) h -> p t h", p=128)
            with tc.tile_pool(name="p5ps", bufs=2, space="PSUM") as p5ps:
                for cq in range(4):
                    for jj in range(4):
                        j = cq * 4 + jj
                        tr = p5ps.tile([128, 65], F16, tag="tr", name=f"tr{j}")
                        nc.tensor.transpose(
                            tr[:], oasb[:, j * 128:(j + 1) * 128], ident[0:65, 0:65])
                        rc = p5sb.tile([128, 1], F32, tag="rc", name=f"rc{j}")
                        nc.vector.reciprocal(rc[:], tr[:, 64:65])
                        nc.vector.tensor_scalar(
                            osb_all[:, j, :], tr[:, 0:64], rc[:], None,
                            op0=mybir.AluOpType.mult)
                    nc.sync.dma_start(
                        out=out_r[:, cq * 4:(cq + 1) * 4, :],
                        in_=osb_all[:, cq * 4:(cq + 1) * 4, :])

    nc.finalize()
    return nc


def get_nc():
    if "nc" not in _CACHE:
        _CACHE["nc"] = _build_nc()
    return _CACHE["nc"]


def make_in_maps(inputs):
    q = np.asarray(inputs["query"], np.float32)
    k = np.asarray(inputs["key_"], np.float32)
    v = np.asarray(inputs["value"], np.float32)
    wq = np.ascontiguousarray(np.asarray(inputs["Wq"], np.float32))
    wk = np.ascontiguousarray(np.asarray(inputs["Wk"], np.float32))
    wv = np.ascontiguousarray(np.asarray(inputs["Wv"], np.float32))
    bq = np.ascontiguousarray(np.asarray(inputs["bq"], np.float32).reshape(H, 1))
    bv = np.ascontiguousarray(np.asarray(inputs["bv"], np.float32).reshape(H, 1))
    in_maps = []
    for b in range(B):
        in_maps.append({
            "xqt": np.ascontiguousarray(q[b].T),
            "xkt": np.ascontiguousarray(k[b].T),
            "xvt": np.ascontiguousarray(v[b].T),
            "wq": wq, "wk": wk, "wv": wv,
            "bq": bq, "bv": bv,
        })
    return in_maps


def kernel(**inputs):
    nc = get_nc()
    in_maps = make_in_maps(inputs)
    res = run_bass_kernel_spmd(nc, in_maps, list(range(B)))
    return np.stack([res.results[b]["out"] for b in range(B)], axis=0)
